# revision 17
# baseline (speedup 1.0000x reference)
"""Distributed causal multi-head attention block for 8 TRN2 NeuronCores.

Sharding: core i -> (batch b = i//2, head-half hh = i%2).  Each core computes
attention for 6 of the 12 heads of one batch element, then a row-sharded
c_proj (its 384 input channels -> full 768 outputs, partial sums).  The
host sums the two partial projections per batch and adds the constant
vector (b_proj + W_proj @ b_v); softmax rows sum to 1 so the v-bias
contributes exactly +b_v per channel and commutes through c_proj.

Engine plan (per core):
  ACT (scalar) : exp ONLY (the softmax exponentials are the wall --
                 ~12.6M elements at 1 elem/cycle/lane @1.2GHz)
  PE  (tensor) : qkv proj, S^T = K@Q^T (two heads row-tiled concurrently
                 via base-partition 0/64), O^T = [V|1]^T@P^T, c_proj
  DVE (vector) : qk bias add, V copy, causal mask (128-wide diagonal
                 square only), softmax normalize, c_proj PSUM->SBUF copy
  GPSIMD       : partition-broadcast of the softmax reciprocal

All projection matmuls are emitted as micro-fillers (<=3 matmuls each)
interleaved one-per-kt into the attention loop so the strict-FIFO PE
queue fills its gaps while ACT streams exps back-to-back.

All per-tensor SBUF residents use a packed [128, n_tiles*W] layout so
each load is one large contiguous DMA (host pre-transposes accordingly).
"""

import sys

sys.path.insert(0, "/opt/trn_rl_repo")

import numpy as np
import ml_dtypes

import concourse.bass as bass
import concourse.bacc as bacc
import concourse.mybir as mybir
import concourse.tile as tile
from concourse.bass_utils import run_bass_kernel_spmd

BF16 = mybir.dt.bfloat16
F32 = mybir.dt.float32
AF = mybir.ActivationFunctionType
ALU = mybir.AluOpType

B, T, C, H, HD = 4, 2048, 768, 12, 64
NCORES = 8
HH = 6              # heads per core
CH = HH * HD        # 384 channels per core
NCT = C // 128      # 6 contraction tiles over C
NQC = T // 512      # 4 query chunks
VW = 65             # per-head V block width (64 dims + ones column)


def _build_graph():
    nc = bacc.Bacc("TRN2", target_bir_lowering=False)

    # packed layouts: [128, n_tiles * width], tile ct at cols [ct*W,(ct+1)*W)
    xT = nc.declare_dram_parameter("xT", [128, NCT * T], BF16, isOutput=False)
    wqkT = nc.declare_dram_parameter("wqkT", [128, NCT * 2 * CH], BF16, isOutput=False)
    bqk = nc.declare_dram_parameter("bqk", [128, 2 * CH // 128], F32, isOutput=False)
    wvT = nc.declare_dram_parameter("wvT", [128, NCT * CH], BF16, isOutput=False)
    wpT = nc.declare_dram_parameter("wpT", [128, (CH // 128) * C], BF16, isOutput=False)
    tri = nc.declare_dram_parameter("tri", [128, 256], BF16, isOutput=False)
    out = nc.declare_dram_parameter("out", [C, T], BF16, isOutput=True)

    with tile.TileContext(nc) as tc:
        with (
            tc.tile_pool(name="weights", bufs=1) as wpool,
            tc.tile_pool(name="acts", bufs=1) as apool,
            tc.tile_pool(name="ps2", bufs=2, space="PSUM") as ps2,
            tc.tile_pool(name="pacc", bufs=2, space="PSUM") as pacc,
            tc.tile_pool(name="pmm", bufs=2, space="PSUM") as pmm,
            tc.tile_pool(name="ptile", bufs=8) as ppool,
            tc.tile_pool(name="small", bufs=6) as spool,
            tc.tile_pool(name="ostage", bufs=6) as opool,
        ):
            xT_b = wpool.tile([128, NCT * T], BF16, tag="xT")
            wqkT_b = wpool.tile([128, NCT * 2 * CH], BF16, tag="wqk")
            wvT_b = wpool.tile([128, NCT * CH], BF16, tag="wv")
            wpT_b = wpool.tile([128, (CH // 128) * C], BF16, tag="wp")
            bqk_s = wpool.tile([128, 2 * CH // 128], F32, tag="bqk")
            tri_s = wpool.tile([128, 256], BF16, tag="tri")

            # xT packed tcn-major: [p, tcn*3072 + ct*512 + t'] so each
            # 512-token column chunk is one contiguous 6144B/partition DMA
            def xsl(ct, tcn):
                o = tcn * (NCT * 512) + ct * 512
                return xT_b[:, o:o + 512]

            def xvsl(ct, tt):
                o = (tt // 4) * (NCT * 512) + ct * 512 + (tt % 4) * 128
                return xT_b[:, o:o + 128]

            def wqks(ct):
                return wqkT_b[:, ct * 2 * CH:(ct + 1) * 2 * CH]

            def wvs(ct):
                return wvT_b[:, ct * CH:(ct + 1) * CH]

            def wps(ct):
                return wpT_b[:, ct * C:(ct + 1) * C]

            qkT_s = [apool.tile([128, T], BF16, tag=f"qkT{i}", name=f"qkT{i}") for i in range(NCT)]
            v_s = [apool.tile([128, HH * VW], BF16, tag=f"v{i}", name=f"v{i}") for i in range(16)]
            yT_s = [apool.tile([128, T], BF16, tag=f"yT{i}", name=f"yT{i}") for i in range(CH // 128)]

            # ---- loads: one large contiguous DMA per tensor/chunk; only
            # the critical set (wqkT + first xT chunk + biases) is issued
            # first so it gets the full HBM bandwidth ----
            CW = NCT * 512
            nc.sync.dma_start(bqk_s[:], bqk[:, :])
            nc.scalar.dma_start(tri_s[:], tri[:, :])
            nc.sync.dma_start(wqkT_b[:], wqkT[:, :])
            nc.scalar.dma_start(xT_b[:, 0:CW], xT[:, 0:CW])
            nc.sync.dma_start(xT_b[:, CW:2 * CW], xT[:, CW:2 * CW])
            nc.scalar.dma_start(wvT_b[:], wvT[:, :])
            nc.sync.dma_start(xT_b[:, 2 * CW:3 * CW], xT[:, 2 * CW:3 * CW])
            nc.scalar.dma_start(xT_b[:, 3 * CW:4 * CW], xT[:, 3 * CW:4 * CW])
            nc.sync.dma_start(wpT_b[:], wpT[:, :])

            # ones columns of the V blocks never change: set them once
            for tt in range(16):
                v3 = v_s[tt][:].rearrange("p (h w) -> p h w", w=VW)
                nc.vector.memset(v3[:, :, 64:65], 1.0)

            tri3 = tri_s[:].rearrange("p (c q) -> p c q", c=2)

            # ---- micro-fillers: <=3 matmuls each, popped one per kt ----
            fillers = []

            def pop_filler():
                if fillers:
                    fillers.pop(0)()

            def flush_fillers():
                while fillers:
                    fillers.pop(0)()

            def qk_micros(ot, tcn):
                st = {}

                def m1(ot=ot, tcn=tcn):
                    ps = pmm.tile([128, 512], F32, tag="mm", name="ps")
                    st["ps"] = ps
                    for ct in range(3):
                        nc.tensor.matmul(
                            ps[:],
                            lhsT=wqks(ct)[:, ot * 128:(ot + 1) * 128],
                            rhs=xsl(ct, tcn),
                            start=(ct == 0), stop=False,
                        )

                def m2(ot=ot, tcn=tcn):
                    ps = st["ps"]
                    for ct in range(3, NCT):
                        nc.tensor.matmul(
                            ps[:],
                            lhsT=wqks(ct)[:, ot * 128:(ot + 1) * 128],
                            rhs=xsl(ct, tcn),
                            start=False, stop=(ct == NCT - 1),
                        )
                    nc.vector.tensor_scalar_add(
                        qkT_s[ot][:, tcn * 512:(tcn + 1) * 512], ps[:],
                        bqk_s[:, ot:ot + 1],
                    )

                return [m1, m2]

            def v_micros(tt):
                st = {}

                def m1(tt=tt):
                    ps = pmm.tile([128, CH], F32, tag="mm", name="ps")
                    st["ps"] = ps
                    for ct in range(3):
                        nc.tensor.matmul(
                            ps[:],
                            lhsT=xvsl(ct, tt),
                            rhs=wvs(ct)[:],
                            start=(ct == 0), stop=False,
                        )

                def m2(tt=tt):
                    ps = st["ps"]
                    for ct in range(3, NCT):
                        nc.tensor.matmul(
                            ps[:],
                            lhsT=xvsl(ct, tt),
                            rhs=wvs(ct)[:],
                            start=False, stop=(ct == NCT - 1),
                        )
                    v3 = v_s[tt][:].rearrange("p (h w) -> p h w", w=VW)
                    nc.vector.tensor_copy(
                        v3[:, :, 0:64], ps[:].rearrange("p (h d) -> p h d", d=64)
                    )

                return [m1, m2]

            def proj_micro(ot, tcn):
                def m(ot=ot, tcn=tcn):
                    ps = pmm.tile([128, 512], F32, tag="mm", name="ps")
                    for ct in range(CH // 128):
                        nc.tensor.matmul(
                            ps[:],
                            lhsT=wps(ct)[:, ot * 128:(ot + 1) * 128],
                            rhs=yT_s[ct][:, tcn * 512:(tcn + 1) * 512],
                            start=(ct == 0),
                            stop=(ct == CH // 128 - 1),
                        )
                    so = opool.tile([128, 512], BF16, tag="so", name="so")
                    nc.vector.tensor_copy(so[:], ps[:])
                    nc.sync.dma_start(
                        out[ot * 128:(ot + 1) * 128, tcn * 512:(tcn + 1) * 512],
                        so[:],
                    )
                return [m]

            # final query chunk's c_proj: the ct=0/1 partials only need
            # hp=0/1's y (ready early) -> staged to SBUF inside the last
            # attention block; the tail does just ct=2 + fused add + store
            soA = [apool.tile([128, 512], F32, tag=f"soA{i}", name=f"soA{i}")
                   for i in range(NCT)]

            def projA_micro(ot, tcn):
                def m(ot=ot, tcn=tcn):
                    ps = pmm.tile([128, 512], F32, tag="mm", name="ps")
                    for ct in range(2):
                        nc.tensor.matmul(
                            ps[:],
                            lhsT=wps(ct)[:, ot * 128:(ot + 1) * 128],
                            rhs=yT_s[ct][:, tcn * 512:(tcn + 1) * 512],
                            start=(ct == 0), stop=(ct == 1),
                        )
                    nc.vector.tensor_copy(soA[ot][:], ps[:])
                return [m]

            def projB_tail(ot, tcn):
                ps = pmm.tile([128, 512], F32, tag="mm", name="ps")
                nc.tensor.matmul(
                    ps[:],
                    lhsT=wps(2)[:, ot * 128:(ot + 1) * 128],
                    rhs=yT_s[2][:, tcn * 512:(tcn + 1) * 512],
                    start=True, stop=True,
                )
                so = opool.tile([128, 512], BF16, tag="so", name="so")
                nc.vector.scalar_tensor_tensor(
                    so[:], ps[:], 1.0, soA[ot][:], ALU.mult, ALU.add,
                )
                nc.sync.dma_start(
                    out[ot * 128:(ot + 1) * 128, tcn * 512:(tcn + 1) * 512],
                    so[:],
                )

            def attention_block(hp, qc, pops_per_kt=1):
                qt = hp          # Q^T rows for heads 2hp,2hp+1 live in tile hp
                ktile = 3 + hp
                o_acc = [pacc.tile([65, 512], F32, tag="oacc", name="oacc")
                         for _ in range(2)]
                nkt = 4 * (qc + 1)
                pend = []        # software pipeline: O(kt) issued after S(kt+2)

                def emit_o(p2, kt, w):
                    for hi in range(2):
                        h = 2 * hp + hi
                        nc.tensor.matmul(
                            o_acc[hi][:, w:512],
                            lhsT=v_s[kt][:, h * VW:(h + 1) * VW],
                            rhs=p2[:, hi * 512 + w:(hi + 1) * 512],
                            start=(kt == 0),
                            stop=(kt == nkt - 1),
                        )

                for kt in range(nkt):
                    j = kt - 4 * qc
                    # columns q < j*128 of a diagonal block are fully
                    # masked: S, exp and O all skip them (the first O
                    # matmul, start=True, is always full width)
                    w = j * 128 if j >= 1 else 0
                    s2 = ps2.tile([128, 1024], F32, tag="s2", name="s2")
                    for hi in range(2):
                        base = hi * 64
                        nc.tensor.matmul(
                            s2[:, hi * 512 + w:(hi + 1) * 512],
                            lhsT=qkT_s[ktile][base:base + 64,
                                              kt * 128:(kt + 1) * 128],
                            rhs=qkT_s[qt][base:base + 64,
                                          qc * 512 + w:(qc + 1) * 512],
                            start=True, stop=True,
                        )
                    if len(pend) >= 2:
                        emit_o(*pend.pop(0))
                    p2 = ppool.tile([128, 1024], BF16, tag="pt", name="p2")
                    p3 = p2[:].rearrange("p (c q) -> p c q", c=2)
                    s3 = s2[:].rearrange("p (c q) -> p c q", c=2)
                    nc.scalar.activation(
                        p3[:, :, w:512], s3[:, :, w:512], AF.Exp, scale=0.125,
                    )
                    if j >= 0:
                        # only the 128-wide diagonal square is partially
                        # masked; its pattern is the same triangle for all j
                        nc.vector.tensor_mul(
                            p3[:, :, j * 128:(j + 1) * 128],
                            p3[:, :, j * 128:(j + 1) * 128],
                            tri3[:, :, :],
                        )
                    pend.append((p2, kt, w))
                    for _ in range(pops_per_kt):
                        pop_filler()
                # remaining fillers may include V copies that the trailing
                # O matmuls read: flush them first
                flush_fillers()
                while pend:
                    emit_o(*pend.pop(0))
                # normalize -> y^T (unbiased; v-bias folded into host add).
                # copy O out of PSUM fast (frees the banks for the next
                # block), chain recip+broadcast off the DVE queue, and
                # RETURN the final multiplies as fillers for the next
                # block: the gpsimd-broadcast latency then never stalls
                # the DVE FIFO.  the denominator row goes to a partition-0
                # SBUF tile first: the custom-DVE reciprocal must read
                # partition-aligned SBUF on hardware.
                muls = []
                for hi in range(2):
                    base = hi * 64
                    dn = spool.tile([1, 512], F32, tag="dn", name="dn")
                    nc.vector.tensor_copy(dn[:], o_acc[hi][64:65, :])
                    ob = spool.tile([64, 512], F32, tag="ob", name="ob")
                    nc.vector.tensor_copy(ob[:], o_acc[hi][0:64, :])
                    rn = spool.tile([1, 512], F32, tag="rn", name="rn")
                    nc.vector.reciprocal_approx_fast(rn[:], dn[:])
                    rc = spool.tile([64, 512], F32, tag="rc", name="rc")
                    nc.gpsimd.partition_broadcast(rc[:], rn[:], channels=64)

                    def mul(hi=hi, ob=ob, rc=rc, base=base, hp=hp, qc=qc):
                        ysl = yT_s[hp][base:base + 64,
                                       qc * 512:(qc + 1) * 512]
                        nc.vector.tensor_mul(ysl, ob[:], rc[:])
                    muls.append(mul)
                return muls

            # ---- emission schedule (engine queues are strict FIFO, so
            # every filler a block's first instructions depend on must be
            # flushed before the block starts; fillers inside a block may
            # only depend on data ready by their pop slot) ----
            for m in qk_micros(0, 0) + qk_micros(3, 0):
                m()
            fillers += v_micros(0) + v_micros(1) + v_micros(2) + v_micros(3)
            fillers += qk_micros(0, 1)
            mf = attention_block(0, 0, pops_per_kt=2)    # 4 kts x 2 pops
            fillers += mf + qk_micros(3, 1) + v_micros(4) + v_micros(5) \
                + v_micros(6) + v_micros(7) + qk_micros(0, 2)
            mf = attention_block(0, 1, pops_per_kt=2)    # 8 kts x 2 pops
            fillers += mf + qk_micros(3, 2) + v_micros(8) + v_micros(9) \
                + v_micros(10) + v_micros(11) + qk_micros(0, 3)
            mf = attention_block(0, 2)           # 12 kts
            fillers += mf + qk_micros(3, 3) + v_micros(12) + v_micros(13) \
                + v_micros(14) + v_micros(15) + qk_micros(1, 0) + qk_micros(4, 0)
            mf = attention_block(0, 3)           # 16 kts
            fillers += mf + qk_micros(1, 1) + qk_micros(4, 1)
            mf = attention_block(1, 0)
            fillers += mf + qk_micros(1, 2) + qk_micros(4, 2)
            mf = attention_block(1, 1)
            fillers += mf + qk_micros(1, 3) + qk_micros(4, 3)
            mf = attention_block(1, 2)
            fillers += mf + qk_micros(2, 0) + qk_micros(5, 0)
            mf = attention_block(1, 3)
            fillers += mf + qk_micros(2, 1) + qk_micros(5, 1)
            mf = attention_block(2, 0)
            fillers += mf + qk_micros(2, 2) + qk_micros(5, 2)
            for ot in range(2):
                fillers += proj_micro(ot, 0)
            mf = attention_block(2, 1)
            fillers += mf + qk_micros(2, 3) + qk_micros(5, 3)
            for ot in range(2, NCT):
                fillers += proj_micro(ot, 0)
            for ot in range(4):
                fillers += proj_micro(ot, 1)
            mf = attention_block(2, 2)
            fillers += mf
            for ot in range(4, NCT):
                fillers += proj_micro(ot, 1)
            for ot in range(NCT):
                fillers += proj_micro(ot, 2)
            for ot in range(NCT):
                fillers += projA_micro(ot, 3)
            mf = attention_block(2, 3)
            for m in mf:
                m()
            for ot in range(NCT):
                projB_tail(ot, 3)
    nc.compile()
    return nc


_CACHE: dict = {}


def _get_graph():
    if "nc" not in _CACHE:
        _CACHE["nc"] = _build_graph()
    return _CACHE["nc"]


def _bf16(a):
    return np.ascontiguousarray(a.astype(ml_dtypes.bfloat16))


def _pack(a):
    """[n*128, W] -> [128, n*W]: row ct*128+p, col w -> row p, col ct*W+w."""
    n = a.shape[0] // 128
    return a.reshape(n, 128, a.shape[1]).transpose(1, 0, 2).reshape(128, -1)


def _pack_x(a):
    """[768, 2048] -> [128, tcn*3072 + ct*512 + t'] (tcn-major chunks)."""
    arr = a.reshape(NCT, 128, NQC, 512)          # [ct, p, tcn, t']
    return arr.transpose(1, 2, 0, 3).reshape(128, -1)


def _make_tri():
    k = np.arange(128)[:, None]
    q = np.arange(128)[None, :]
    pat = (q >= k).astype(np.float32)
    return _bf16(np.concatenate([pat, pat], axis=1))  # [128, 256], dup for c=2


def _prepare_in_maps(x, W_attn, b_attn, W_proj):
    tri = _make_tri()
    in_maps = []
    for core in range(NCORES):
        b, hh = core // 2, core % 2
        sl = slice(hh * CH, (hh + 1) * CH)
        wq = W_attn[0 * C:1 * C][sl]          # [384, 768]
        wk = W_attn[1 * C:2 * C][sl]
        wv = W_attn[2 * C:3 * C][sl]
        bq = b_attn[0 * C:1 * C][sl]
        bk = b_attn[1 * C:2 * C][sl]
        in_maps.append({
            "xT": _bf16(_pack_x(x[b].T)),                          # [128, 4*3072]
            "wqkT": _bf16(_pack(np.concatenate([wq, wk], 0).T)),   # [128, 6*768]
            "bqk": np.ascontiguousarray(
                np.concatenate([bq, bk]).reshape(-1, 128).T),      # [128, 6]
            "wvT": _bf16(_pack(wv.T)),                             # [128, 6*384]
            "wpT": _bf16(_pack(W_proj[:, sl].T)),                  # [128, 3*768]
            "tri": tri,
        })
    return in_maps


def _unshard(outs, W_proj, b_attn, b_proj):
    bv = b_attn[2 * C:3 * C]
    const = W_proj @ bv + b_proj                     # [768]
    y = np.empty((B, T, C), np.float32)
    for b in range(B):
        acc = (np.asarray(outs[2 * b]["out"], np.float32)
               + np.asarray(outs[2 * b + 1]["out"], np.float32))
        y[b] = acc.T + const
    return y


def run(x, W_attn, b_attn, W_proj, b_proj, **spmd_kwargs):
    x = np.asarray(x, np.float32)
    W_attn = np.asarray(W_attn, np.float32)
    b_attn = np.asarray(b_attn, np.float32)
    W_proj = np.asarray(W_proj, np.float32)
    b_proj = np.asarray(b_proj, np.float32)
    in_maps = _prepare_in_maps(x, W_attn, b_attn, W_proj)
    nc = _get_graph()
    res = run_bass_kernel_spmd(
        nc, in_maps, core_ids=list(range(NCORES)), **spmd_kwargs
    )
    return _unshard(res.results, W_proj, b_attn, b_proj), res


def kernel(x, W_attn, b_attn, W_proj, b_proj):
    y, _ = run(x, W_attn, b_attn, W_proj, b_proj)
    return y


# revision 20
# speedup vs baseline: 1.0035x; 1.0035x over previous
"""Distributed causal multi-head attention block for 8 TRN2 NeuronCores.

Sharding: core i -> (batch b = i//2, head-half hh = i%2).  Each core computes
attention for 6 of the 12 heads of one batch element, then a row-sharded
c_proj (its 384 input channels -> full 768 outputs, partial sums).  The
host sums the two partial projections per batch and adds the constant
vector (b_proj + W_proj @ b_v); softmax rows sum to 1 so the v-bias
contributes exactly +b_v per channel and commutes through c_proj.

Engine plan (per core):
  ACT (scalar) : exp ONLY (the softmax exponentials are the wall --
                 ~12.6M elements at 1 elem/cycle/lane @1.2GHz)
  PE  (tensor) : qkv proj, S^T = K@Q^T (two heads row-tiled concurrently
                 via base-partition 0/64), O^T = [V|1]^T@P^T, c_proj
  DVE (vector) : qk bias add, V copy, causal mask (128-wide diagonal
                 square only), softmax normalize, c_proj PSUM->SBUF copy
  GPSIMD       : partition-broadcast of the softmax reciprocal

All projection matmuls are emitted as micro-fillers (<=3 matmuls each)
popped into the attention kt-loop so the strict-FIFO PE queue fills its
gaps while ACT streams exps back-to-back.  Each block's trailing O
matmuls + normalization are emitted AFTER the next block's first
S/exp step, removing the inter-block ACT bubble; the normalize multiply
itself is deferred into the next block's filler stream so the gpsimd
broadcast latency never stalls the DVE FIFO.
"""

import sys

sys.path.insert(0, "/opt/trn_rl_repo")

import numpy as np
import ml_dtypes

import concourse.bass as bass
import concourse.bacc as bacc
import concourse.mybir as mybir
import concourse.tile as tile
from concourse.bass_utils import run_bass_kernel_spmd

BF16 = mybir.dt.bfloat16
F32 = mybir.dt.float32
AF = mybir.ActivationFunctionType
ALU = mybir.AluOpType

B, T, C, H, HD = 4, 2048, 768, 12, 64
NCORES = 8
HH = 6              # heads per core
CH = HH * HD        # 384 channels per core
NCT = C // 128      # 6 contraction tiles over C
NQC = T // 512      # 4 query chunks
VW = 65             # per-head V block width (64 dims + ones column)


def _build_graph():
    nc = bacc.Bacc("TRN2", target_bir_lowering=False)

    # packed layouts: [128, ...]; xT is tcn-major so each 512-token column
    # chunk is one contiguous 6144B-per-partition DMA
    xT = nc.declare_dram_parameter("xT", [128, NCT * T], BF16, isOutput=False)
    wqkT = nc.declare_dram_parameter("wqkT", [128, NCT * 2 * CH], BF16, isOutput=False)
    bqk = nc.declare_dram_parameter("bqk", [128, 2 * CH // 128], F32, isOutput=False)
    wvT = nc.declare_dram_parameter("wvT", [128, NCT * CH], BF16, isOutput=False)
    wpT = nc.declare_dram_parameter("wpT", [128, (CH // 128) * C], BF16, isOutput=False)
    tri = nc.declare_dram_parameter("tri", [128, 256], BF16, isOutput=False)
    out = nc.declare_dram_parameter("out", [C, T], BF16, isOutput=True)

    with tile.TileContext(nc) as tc:
        with (
            tc.tile_pool(name="weights", bufs=1) as wpool,
            tc.tile_pool(name="acts", bufs=1) as apool,
            tc.tile_pool(name="ps2", bufs=2, space="PSUM") as ps2,
            tc.tile_pool(name="pacc", bufs=2, space="PSUM") as pacc,
            tc.tile_pool(name="pmm", bufs=2, space="PSUM") as pmm,
            tc.tile_pool(name="ptile", bufs=8) as ppool,
            tc.tile_pool(name="small", bufs=6) as spool,
            tc.tile_pool(name="ostage", bufs=6) as opool,
        ):
            xT_b = wpool.tile([128, NCT * T], BF16, tag="xT")
            wqkT_b = wpool.tile([128, NCT * 2 * CH], BF16, tag="wqk")
            wvT_b = wpool.tile([128, NCT * CH], BF16, tag="wv")
            wpT_b = wpool.tile([128, (CH // 128) * C], BF16, tag="wp")
            bqk_s = wpool.tile([128, 2 * CH // 128], F32, tag="bqk")
            tri_s = wpool.tile([128, 256], BF16, tag="tri")
            ones64 = wpool.tile([1, 64], F32, tag="ones64")

            def xsl(ct, tcn):
                o = tcn * (NCT * 512) + ct * 512
                return xT_b[:, o:o + 512]

            def xvsl(ct, tt):
                o = (tt // 4) * (NCT * 512) + ct * 512 + (tt % 4) * 128
                return xT_b[:, o:o + 128]

            def wqks(ct):
                return wqkT_b[:, ct * 2 * CH:(ct + 1) * 2 * CH]

            def wvs(ct):
                return wvT_b[:, ct * CH:(ct + 1) * CH]

            def wps(ct):
                return wpT_b[:, ct * C:(ct + 1) * C]

            qkT_s = [apool.tile([128, T], BF16, tag=f"qkT{i}", name=f"qkT{i}") for i in range(NCT)]
            v_s = [apool.tile([128, HH * VW], BF16, tag=f"v{i}", name=f"v{i}") for i in range(16)]
            yT_s = [apool.tile([128, T], BF16, tag=f"yT{i}", name=f"yT{i}") for i in range(CH // 128)]
            soA = [apool.tile([128, 512], F32, tag=f"soA{i}", name=f"soA{i}")
                   for i in range(NCT)]

            # ---- loads: contiguous DMAs, critical halves first ----
            CW = NCT * 512
            HQ = NCT * CH          # half of wqkT (ct 0..2)
            nc.sync.dma_start(bqk_s[:], bqk[:, :])
            nc.scalar.dma_start(tri_s[:], tri[:, :])
            nc.sync.dma_start(wqkT_b[:, 0:HQ], wqkT[:, 0:HQ])
            nc.scalar.dma_start(xT_b[:, 0:CW // 2], xT[:, 0:CW // 2])
            nc.sync.dma_start(wqkT_b[:, HQ:2 * HQ], wqkT[:, HQ:2 * HQ])
            nc.scalar.dma_start(xT_b[:, CW // 2:CW], xT[:, CW // 2:CW])
            nc.sync.dma_start(xT_b[:, CW:2 * CW], xT[:, CW:2 * CW])
            nc.scalar.dma_start(wvT_b[:], wvT[:, :])
            nc.sync.dma_start(xT_b[:, 2 * CW:3 * CW], xT[:, 2 * CW:3 * CW])
            nc.scalar.dma_start(xT_b[:, 3 * CW:4 * CW], xT[:, 3 * CW:4 * CW])
            nc.sync.dma_start(wpT_b[:], wpT[:, :])

            # ones columns of the V blocks never change: set them once
            for tt in range(16):
                v3 = v_s[tt][:].rearrange("p (h w) -> p h w", w=VW)
                nc.vector.memset(v3[:, :, 64:65], 1.0)
            nc.vector.memset(ones64[:], 1.0)

            tri3 = tri_s[:].rearrange("p (c q) -> p c q", c=2)

            # PE warm-up: ~5us of junk matmuls on the (tiny, early) tri
            # tile while the big DMAs stream, so the HAM clock-gate is at
            # 8/8 when the real projections start
            warm = pmm.tile([128, 512], F32, tag="mm", name="warm")
            for i in range(24):
                nc.tensor.matmul(
                    warm[:, 0:256], lhsT=tri_s[:, 0:128], rhs=tri_s[:, 0:256],
                    start=(i == 0), stop=(i == 23),
                )

            # ---- micro-fillers ----
            fillers = []

            def pop_filler():
                if fillers:
                    fillers.pop(0)()

            def flush_fillers():
                while fillers:
                    fillers.pop(0)()

            def qk_micros(ot, tcn):
                st = {}

                def m1(ot=ot, tcn=tcn):
                    ps = pmm.tile([128, 512], F32, tag="mm", name="ps")
                    st["ps"] = ps
                    for ct in range(3):
                        nc.tensor.matmul(
                            ps[:],
                            lhsT=wqks(ct)[:, ot * 128:(ot + 1) * 128],
                            rhs=xsl(ct, tcn),
                            start=(ct == 0), stop=False,
                        )

                def m2(ot=ot, tcn=tcn):
                    ps = st["ps"]
                    for ct in range(3, NCT):
                        nc.tensor.matmul(
                            ps[:],
                            lhsT=wqks(ct)[:, ot * 128:(ot + 1) * 128],
                            rhs=xsl(ct, tcn),
                            start=False, stop=(ct == NCT - 1),
                        )
                    nc.vector.tensor_scalar_add(
                        qkT_s[ot][:, tcn * 512:(tcn + 1) * 512], ps[:],
                        bqk_s[:, ot:ot + 1],
                    )

                return [m1, m2]

            def v_micros(tt):
                st = {}

                def m1(tt=tt):
                    ps = pmm.tile([128, CH], F32, tag="mm", name="ps")
                    st["ps"] = ps
                    for ct in range(3):
                        nc.tensor.matmul(
                            ps[:],
                            lhsT=xvsl(ct, tt),
                            rhs=wvs(ct)[:],
                            start=(ct == 0), stop=False,
                        )

                def m2(tt=tt):
                    ps = st["ps"]
                    for ct in range(3, NCT):
                        nc.tensor.matmul(
                            ps[:],
                            lhsT=xvsl(ct, tt),
                            rhs=wvs(ct)[:],
                            start=False, stop=(ct == NCT - 1),
                        )
                    v3 = v_s[tt][:].rearrange("p (h w) -> p h w", w=VW)
                    nc.vector.tensor_copy(
                        v3[:, :, 0:64], ps[:].rearrange("p (h d) -> p h d", d=64)
                    )

                return [m1, m2]

            def proj_micro(ot, tcn):
                def m(ot=ot, tcn=tcn):
                    ps = pmm.tile([128, 512], F32, tag="mm", name="ps")
                    for ct in range(CH // 128):
                        nc.tensor.matmul(
                            ps[:],
                            lhsT=wps(ct)[:, ot * 128:(ot + 1) * 128],
                            rhs=yT_s[ct][:, tcn * 512:(tcn + 1) * 512],
                            start=(ct == 0),
                            stop=(ct == CH // 128 - 1),
                        )
                    so = opool.tile([128, 512], BF16, tag="so", name="so")
                    nc.vector.tensor_copy(so[:], ps[:])
                    nc.sync.dma_start(
                        out[ot * 128:(ot + 1) * 128, tcn * 512:(tcn + 1) * 512],
                        so[:],
                    )
                return [m]

            # final query chunk's c_proj: ct=0/1 partials (hp=0/1's y, ready
            # early) staged to SBUF inside the last block; the tail is just
            # ct=2 + fused add + store
            def projA_micro(ot, tcn):
                def m(ot=ot, tcn=tcn):
                    ps = pmm.tile([128, 512], F32, tag="mm", name="ps")
                    for ct in range(2):
                        nc.tensor.matmul(
                            ps[:],
                            lhsT=wps(ct)[:, ot * 128:(ot + 1) * 128],
                            rhs=yT_s[ct][:, tcn * 512:(tcn + 1) * 512],
                            start=(ct == 0), stop=(ct == 1),
                        )
                    nc.vector.tensor_copy(soA[ot][:], ps[:])
                return [m]

            def projB_tail(ot, tcn):
                ps = pmm.tile([128, 512], F32, tag="mm", name="ps")
                nc.tensor.matmul(
                    ps[:],
                    lhsT=wps(2)[:, ot * 128:(ot + 1) * 128],
                    rhs=yT_s[2][:, tcn * 512:(tcn + 1) * 512],
                    start=True, stop=True,
                )
                so = opool.tile([128, 512], BF16, tag="so", name="so")
                nc.vector.scalar_tensor_tensor(
                    so[:], ps[:], 1.0, soA[ot][:], ALU.mult, ALU.add,
                )
                nc.sync.dma_start(
                    out[ot * 128:(ot + 1) * 128, tcn * 512:(tcn + 1) * 512],
                    so[:],
                )

            def attn_steps(hp, qc, pops_per_kt=1):
                qt = hp          # Q^T rows for heads 2hp,2hp+1 live in tile hp
                ktile = 3 + hp
                nkt = 4 * (qc + 1)
                st = {"o": None, "pend": []}

                def emit_o(p2, kt, w):
                    for hi in range(2):
                        h = 2 * hp + hi
                        nc.tensor.matmul(
                            st["o"][hi][:, w:512],
                            lhsT=v_s[kt][:, h * VW:(h + 1) * VW],
                            rhs=p2[:, hi * 512 + w:(hi + 1) * 512],
                            start=(kt == 0),
                            stop=(kt == nkt - 1),
                        )

                def step(kt):
                    if kt == 0:
                        st["o"] = [pacc.tile([65, 512], F32, tag="oacc",
                                             name="oacc") for _ in range(2)]
                    j = kt - 4 * qc
                    # columns q < j*128 of a diagonal block are fully
                    # masked: S, exp and O all skip them (the first O
                    # matmul, start=True, is always full width)
                    w = j * 128 if j >= 1 else 0
                    s2 = ps2.tile([128, 1024], F32, tag="s2", name="s2")
                    for hi in range(2):
                        base = hi * 64
                        nc.tensor.matmul(
                            s2[:, hi * 512 + w:(hi + 1) * 512],
                            lhsT=qkT_s[ktile][base:base + 64,
                                              kt * 128:(kt + 1) * 128],
                            rhs=qkT_s[qt][base:base + 64,
                                          qc * 512 + w:(qc + 1) * 512],
                            start=True, stop=True,
                        )
                    if len(st["pend"]) >= 2:
                        emit_o(*st["pend"].pop(0))
                    p2 = ppool.tile([128, 1024], BF16, tag="pt", name="p2")
                    p3 = p2[:].rearrange("p (c q) -> p c q", c=2)
                    s3 = s2[:].rearrange("p (c q) -> p c q", c=2)
                    nc.scalar.activation(
                        p3[:, :, w:512], s3[:, :, w:512], AF.Exp, scale=0.125,
                    )
                    if j >= 0:
                        # only the 128-wide diagonal square is partially
                        # masked; same triangle pattern for every j
                        nc.vector.tensor_mul(
                            p3[:, :, j * 128:(j + 1) * 128],
                            p3[:, :, j * 128:(j + 1) * 128],
                            tri3[:, :, :],
                        )
                    st["pend"].append((p2, kt, w))
                    for _ in range(pops_per_kt):
                        pop_filler()

                def tail(final=False):
                    if final:
                        flush_fillers()
                    while st["pend"]:
                        emit_o(*st["pend"].pop(0))
                    # normalize -> y^T (unbiased; v-bias folded into the
                    # host-side add).  copy O out of PSUM fast, chain
                    # recip+broadcast, and RETURN the multiplies so the
                    # caller can pop them later -- the broadcast latency
                    # then never stalls the DVE FIFO.  the denominator row
                    # goes to a partition-0 SBUF tile first (custom-DVE
                    # reciprocal must read partition-aligned SBUF on HW).
                    muls = []
                    for hi in range(2):
                        base = hi * 64
                        dn = spool.tile([1, 512], F32, tag="dn", name="dn")
                        nc.vector.tensor_copy(dn[:], st["o"][hi][64:65, :])
                        ob = spool.tile([64, 512], F32, tag="ob", name="ob")
                        nc.vector.tensor_copy(ob[:], st["o"][hi][0:64, :])
                        rn = spool.tile([1, 512], F32, tag="rn", name="rn")
                        nc.vector.reciprocal_approx_fast(rn[:], dn[:])
                        if final:
                            # broadcast on the (idle, warm) PE via a K=1
                            # matmul: keeps the tail off the gpsimd queue
                            rcp = pmm.tile([128, 512], F32, tag="mm",
                                           name="rcp")
                            nc.tensor.matmul(
                                rcp[0:64, :], lhsT=ones64[:], rhs=rn[:],
                                start=True, stop=True,
                            )
                            rc = rcp[0:64, :]
                        else:
                            rcs = spool.tile([64, 512], F32, tag="rc",
                                             name="rc")
                            nc.gpsimd.partition_broadcast(
                                rcs[:], rn[:], channels=64)
                            rc = rcs[:]

                        def mul(ob=ob, rc=rc, base=base, hp=hp, qc=qc):
                            ysl = yT_s[hp][base:base + 64,
                                           qc * 512:(qc + 1) * 512]
                            nc.vector.tensor_mul(ysl, ob[:], rc)
                        muls.append(mul)
                    return muls

                return [lambda kt=kt: step(kt) for kt in range(nkt)], tail

            # ---- pipelined emission: block i's trailing Os + normalize
            # are emitted AFTER block i+1's first S/exp step; deferred
            # normalize muls are injected at the filler-queue front.
            # queue budgets/pop positions are hand-verified against each
            # block's dependency deadlines. ----
            for m in qk_micros(0, 0) + qk_micros(3, 0):
                m()

            blocks = [
                # (hp, qc, pops_per_kt, pre-set queue)
                (0, 0, 3, v_micros(0) + v_micros(1) + v_micros(2)
                 + v_micros(3) + qk_micros(0, 1)),
                (0, 1, 2, qk_micros(3, 1) + v_micros(4) + v_micros(5)
                 + v_micros(6) + v_micros(7) + qk_micros(0, 2)),
                (0, 2, 2, qk_micros(3, 2) + qk_micros(0, 3) + v_micros(8)
                 + v_micros(9) + v_micros(10) + v_micros(11)),
                (0, 3, 1, qk_micros(3, 3) + v_micros(12) + v_micros(13)
                 + v_micros(14) + v_micros(15) + qk_micros(1, 0)
                 + qk_micros(4, 0)),
                (1, 0, 2, qk_micros(1, 1) + qk_micros(4, 1)),
                (1, 1, 1, qk_micros(1, 2) + qk_micros(4, 2)),
                (1, 2, 1, qk_micros(1, 3) + qk_micros(4, 3)),
                (1, 3, 1, qk_micros(2, 0) + qk_micros(5, 0)),
                (2, 0, 2, qk_micros(2, 1) + qk_micros(5, 1)),
                (2, 1, 1, qk_micros(2, 2) + qk_micros(5, 2)
                 + proj_micro(0, 0) + proj_micro(1, 0)),
                (2, 2, 1, qk_micros(2, 3) + qk_micros(5, 3)
                 + proj_micro(2, 0) + proj_micro(3, 0) + proj_micro(4, 0)
                 + proj_micro(5, 0) + proj_micro(0, 1) + proj_micro(1, 1)),
                (2, 3, 1, proj_micro(2, 1) + proj_micro(3, 1)
                 + proj_micro(4, 1) + proj_micro(5, 1)
                 + [m for ot in range(NCT) for m in proj_micro(ot, 2)]
                 + [m for ot in range(NCT) for m in projA_micro(ot, 3)]),
            ]

            prev_tail = None
            for (hp, qc, ppk, items) in blocks:
                steps, tail = attn_steps(hp, qc, pops_per_kt=ppk)
                fillers.extend(items)
                steps[0]()
                if prev_tail is not None:
                    fillers[0:0] = prev_tail()
                for s in steps[1:]:
                    s()
                prev_tail = tail
            for m in prev_tail(final=True):
                m()
            for ot in range(NCT):
                projB_tail(ot, 3)
    nc.compile()
    return nc


_CACHE: dict = {}


def _get_graph():
    if "nc" not in _CACHE:
        _CACHE["nc"] = _build_graph()
    return _CACHE["nc"]


def _bf16(a):
    return np.ascontiguousarray(a.astype(ml_dtypes.bfloat16))


def _pack(a):
    """[n*128, W] -> [128, n*W]: row ct*128+p, col w -> row p, col ct*W+w."""
    n = a.shape[0] // 128
    return a.reshape(n, 128, a.shape[1]).transpose(1, 0, 2).reshape(128, -1)


def _pack_x(a):
    """[768, 2048] -> [128, tcn*3072 + ct*512 + t'] (tcn-major chunks)."""
    arr = a.reshape(NCT, 128, NQC, 512)          # [ct, p, tcn, t']
    return arr.transpose(1, 2, 0, 3).reshape(128, -1)


def _make_tri():
    k = np.arange(128)[:, None]
    q = np.arange(128)[None, :]
    pat = (q >= k).astype(np.float32)
    return _bf16(np.concatenate([pat, pat], axis=1))  # [128, 256], dup for c=2


def _prepare_in_maps(x, W_attn, b_attn, W_proj):
    tri = _make_tri()
    in_maps = []
    for core in range(NCORES):
        b, hh = core // 2, core % 2
        sl = slice(hh * CH, (hh + 1) * CH)
        wq = W_attn[0 * C:1 * C][sl]          # [384, 768]
        wk = W_attn[1 * C:2 * C][sl]
        wv = W_attn[2 * C:3 * C][sl]
        bq = b_attn[0 * C:1 * C][sl]
        bk = b_attn[1 * C:2 * C][sl]
        in_maps.append({
            "xT": _bf16(_pack_x(x[b].T)),                          # [128, 4*3072]
            "wqkT": _bf16(_pack(np.concatenate([wq, wk], 0).T)),   # [128, 6*768]
            "bqk": np.ascontiguousarray(
                np.concatenate([bq, bk]).reshape(-1, 128).T),      # [128, 6]
            "wvT": _bf16(_pack(wv.T)),                             # [128, 6*384]
            "wpT": _bf16(_pack(W_proj[:, sl].T)),                  # [128, 3*768]
            "tri": tri,
        })
    return in_maps


def _unshard(outs, W_proj, b_attn, b_proj):
    bv = b_attn[2 * C:3 * C]
    const = W_proj @ bv + b_proj                     # [768]
    y = np.empty((B, T, C), np.float32)
    for b in range(B):
        acc = (np.asarray(outs[2 * b]["out"], np.float32)
               + np.asarray(outs[2 * b + 1]["out"], np.float32))
        y[b] = acc.T + const
    return y


def run(x, W_attn, b_attn, W_proj, b_proj, **spmd_kwargs):
    x = np.asarray(x, np.float32)
    W_attn = np.asarray(W_attn, np.float32)
    b_attn = np.asarray(b_attn, np.float32)
    W_proj = np.asarray(W_proj, np.float32)
    b_proj = np.asarray(b_proj, np.float32)
    in_maps = _prepare_in_maps(x, W_attn, b_attn, W_proj)
    nc = _get_graph()
    res = run_bass_kernel_spmd(
        nc, in_maps, core_ids=list(range(NCORES)), **spmd_kwargs
    )
    return _unshard(res.results, W_proj, b_attn, b_proj), res


def kernel(x, W_attn, b_attn, W_proj, b_proj):
    y, _ = run(x, W_attn, b_attn, W_proj, b_proj)
    return y
